# revision 18
# baseline (speedup 1.0000x reference)
"""Trainium2 Bass kernel for nn_Classification2 (histogram_binning).

matrix[x, y] = -mean((clip1[y] - clip2[x])**2) * 1e13 over D = 3*224*224
             = -(SCALE/D) * (||a_x||^2 + ||b_y||^2 - 2 a_x.b_y)
output[k]    = mean of matrix over diagonals y - x = k - 64, k in [0, 129)

Strategy: data-parallel over D across 8 NeuronCores. The device computes ONLY
the gram partials a@b^T (the O(S^2 D) part); the O(S D) squared norms come
from the full-precision f32 inputs on the host, and the O(S^2) diagonal
binning also runs on the host.

Inputs stream as fp8 e4m3 (1 B/elem; values are N(0,1), |x| < 6 << 240, no
clipping needed). e4m3 noise averages out over the >=64-element diagonal
means: measured end-to-end rel_l2 ~ 1e-5 (gate 2e-2).

v3 vs the tile-based v1 (28.3us): RAW bass, no TileContext. The walrus
compiler appends a fixed epilogue [CoreBarrier; ~51 semaphore clears per
engine, one EVENT_SEMAPHORE each; CoreBarrier; NOTIFY]. Tile's end-of-kernel
drain + barriers + RANGE_CLEAR (~1.4us) sat between the write-back receipt
and that storm; raw bass drops them, starts the input stream ~0.6us earlier
(no tile-entry barrier), and balances the two HWDGE rings so neither idles
(v1 put 62% of bytes on the sync ring).

Robustness: each engine CLEARS the semaphores it waits on as its first
instructions. Initial sem state is normally all-zero (every walrus NEFF ends
by zeroing all 256), but a hiccuped/aborted prior execution can leave
garbage that would let waits pass early and read stale SBUF (observed as
intermittent NaN/stale grams under NTFF profiling). The clears are safe:
every producer's first increment trails the consumer's clear by multiple us
(first chunk DMA completes ~3us after PE clears its wait sems).

The PE HAM activity monitor clock-gates the array to 1.2 GHz after ~1us of
idleness, which would also halve the PE sequencer's throughput through its
~51-clear share of the walrus storm (115 ns/clear throttled vs ~57 warm).
Dependency-free warm-up matmuls bridge t0 -> first chunk, pace fillers
bridge inter-chunk gaps, and N_TAIL post-stream fillers keep the clock at
2.4 GHz from the last real matmul until the pre-storm CoreBarrier (gated by
the write-back's HBM receipt on Sync).

Engine roles (program order per engine):
  sync   : issue 6 input-chunk DMAs (76 f)
  scalar : clear sem_cp; issue 5 input-chunk DMAs (71 f); wait sem_cp;
           issue the 64 KB gram write-back
  tensor : clear sem_sync/sem_scal; 15 warm matmuls (uninitialized SBUF -
           values are irrelevant, ps_w is never read); per chunk: wait
           chunk sem, matmuls, fillers; N_TAIL tail fillers
  vector : clear sem_pe; wait sem_pe; DVE-copy PSUM->SBUF
  gpsimd : unused (its share of the walrus storm runs during the stream)
"""

import sys
from contextlib import ExitStack

sys.path.insert(0, "/opt/trn_rl_repo")

import numpy as np

S = 128
D = 150528  # 3*224*224
N_CORES = 8
DC = D // N_CORES  # 18816 d-values per core
F = DC // S  # 147 contraction chunks of K=128
SCALE = 1.0e13

# Chunk layout: (ring, nf) in global (f-consecutive) order. Ring 0 = sync,
# ring 1 = scalar. The scalar (qAct) ring runs ~0.5-1us behind at equal
# bytes, so it gets 5 fewer f. Both rings end on a small chunk so the two
# final SBUF-receipt latencies overlap instead of serializing.
CHUNKS = [
    (0, 16), (1, 14),
    (0, 20), (1, 20),
    (0, 20), (1, 20),
    (0, 14), (1, 11),
    (0, 6), (1, 4),
    (0, 2),
]
assert sum(nf for _, nf in CHUNKS) == F
assert sum(nf for r, nf in CHUNKS if r == 0) == 78
# Pace fillers: the PE consumes a chunk faster than DMA delivers the next,
# and idle lumps > ~1us let the HAM activity monitor re-throttle the PE
# clock to 1.2 GHz mid-stream.
FILLER = [0, 0, 2, 2, 2, 2, 2, 2, 1, 1, 0]
N_WARM = 15  # garbage matmuls at t0: bridge PE activity until chunk 0 lands
N_TAIL = 9  # post-stream fillers: keep the PE at 2.4 GHz until it reaches
            # the pre-storm CoreBarrier (~2us after the last real matmul),
            # so its ~51 storm clears run at ~58 ns instead of 116 throttled

_NC_CACHE = {}


def _build():
    import concourse.bacc as bacc
    import concourse.mybir as mybir

    f32 = mybir.dt.float32
    fp8 = mybir.dt.float8e4

    nc = bacc.Bacc(num_devices=N_CORES)

    ba_in = nc.dram_tensor("ba", [S, F * 256], fp8, kind="ExternalInput")
    out_t = nc.dram_tensor("out", [S * S], f32, kind="ExternalOutput")

    with (
        nc.sbuf_tensor([S, F * 256], fp8) as ba_sb,
        nc.sbuf_tensor([S, 256], fp8) as warm_sb,
        nc.sbuf_tensor([S, S], f32) as g_sb,
        nc.psum_tensor([S, 256], f32) as ps_w,
        nc.psum_tensor([S, S], f32) as ps,
        ExitStack() as stack,
    ):
        # One semaphore PER CHUNK, waited at >=16. A single per-ring sem
        # with cumulative >=16k waits is UNSOUND: the +16 per DMA comes
        # from 16 independent SDMA engines, so a fast engine racing ahead
        # through later chunks can push the sem past 16k while a slow
        # engine still owes bytes of chunk k (observed as intermittent
        # stale-partition grams, worse under NTFF profiling's asymmetric
        # engine load). Tile's DMAHW0-7 lane rotation exists for this
        # reason.
        chunk_sems = [
            stack.enter_context(nc.semaphore(name=f"sem_chunk{ci}"))
            for ci in range(len(CHUNKS))
        ]
        sem_pe = stack.enter_context(nc.semaphore(name="sem_pe"))
        sem_cp = stack.enter_context(nc.semaphore(name="sem_cp"))
        sem_out = stack.enter_context(nc.semaphore(name="sem_out"))

        # Self-heal: every engine zeroes the sems it will wait on, before
        # any producer can have incremented them (initial state is all-zero
        # after any clean NEFF, but a hiccuped prior execution can leave
        # garbage; first chunk completion trails these clears by ~3us).
        nc.scalar.sem_clear(sem_cp)
        for s in chunk_sems:
            nc.tensor.sem_clear(s)
        nc.vector.sem_clear(sem_pe)

        # All input chunk DMAs issued up-front; each HWDGE ring drains its
        # FIFO in order while the issuing engine moves on.
        f0 = 0
        for ci, (ring, nf) in enumerate(CHUNKS):
            sl = slice(f0 * 256, (f0 + nf) * 256)
            eng = nc.sync if ring == 0 else nc.scalar
            eng.dma_start(ba_sb[:, sl], ba_in[:, sl]).then_inc(
                chunk_sems[ci], 16
            )
            f0 += nf

        # PE warm-up on whatever bytes sit in warm_sb (NaNs land in the
        # scratch ps_w bank, which is never read).
        for _ in range(N_WARM):
            nc.tensor.matmul(
                ps_w[:, :], warm_sb[:, 0:S], warm_sb[:, :], start=True, stop=True
            )

        f0 = 0
        for ci, (ring, nf) in enumerate(CHUNKS):
            nc.tensor.wait_ge(chunk_sems[ci], 16)
            for j in range(nf):
                f = f0 + j
                base = f * 256
                mm = nc.tensor.matmul(
                    ps[:, :],
                    ba_sb[:, base + S : base + 256],  # lhsT = A_f (clip2)
                    ba_sb[:, base : base + S],  # rhs = B_f (clip1)
                    start=(f == 0),
                    stop=(f == F - 1),
                )
                if f == F - 1:
                    mm.then_inc(sem_pe, 1)
            for _ in range(FILLER[ci]):
                nc.tensor.matmul(
                    ps_w[:, :], warm_sb[:, 0:S], warm_sb[:, :],
                    start=True, stop=True,
                )
            f0 += nf
        for _ in range(N_TAIL):
            nc.tensor.matmul(
                ps_w[:, :], warm_sb[:, 0:S], warm_sb[:, :],
                start=True, stop=True,
            )

        # Evacuate on DVE, write back on the ACT ring. Nobody waits for the
        # write-back's HBM receipt: NRT quiesces the DMA queues at NEFF end
        # (validated empirically - stale reads would return the donated
        # all-zero output buffer), and skipping the wait lets the walrus
        # semaphore-clear storm overlap the ~1.4us receipt instead of
        # serializing after it.
        nc.vector.wait_ge(sem_pe, 1)
        nc.vector.tensor_copy(g_sb[:, :], ps[:, :]).then_inc(sem_cp, 1)
        nc.scalar.wait_ge(sem_cp, 1)
        nc.scalar.dma_start(
            out_t[0 : S * S].rearrange("(p y) -> p y", p=S), g_sb[:, :]
        ).then_inc(sem_out, 16)  # walrus codegen requires a sem update; no waiter

    nc.finalize()
    return nc


def _get_nc():
    if "nc" not in _NC_CACHE:
        _NC_CACHE["nc"] = _build()
    return _NC_CACHE["nc"]


def _shards(clip1: np.ndarray, clip2: np.ndarray):
    """Per-core fp8 [S, F*256] tensors: cols [B_f | A_f] per f, where
    value (p, f, x) = clip[x, d0 + f*128 + p]."""
    import ml_dtypes

    fp8 = ml_dtypes.float8_e4m3
    c1 = np.asarray(clip1, dtype=np.float32).reshape(S, D).astype(fp8)
    c2 = np.asarray(clip2, dtype=np.float32).reshape(S, D).astype(fp8)
    maps = []
    for c in range(N_CORES):
        sl = slice(c * DC, (c + 1) * DC)
        bt = c1[:, sl].reshape(S, F, S).transpose(2, 1, 0)  # [p, f, y] moving
        at = c2[:, sl].reshape(S, F, S).transpose(2, 1, 0)  # [p, f, x] weights
        ba = np.empty((S, F, 256), dtype=fp8)
        ba[:, :, 0:S] = bt
        ba[:, :, S:256] = at
        maps.append({"ba": ba.reshape(S, F * 256)})
    return maps


def _combine(results, clip1: np.ndarray, clip2: np.ndarray) -> np.ndarray:
    gram = np.zeros((S, S), dtype=np.float64)
    for r in results:
        gram += np.asarray(r["out"], dtype=np.float64).reshape(S, S)
    c1 = np.asarray(clip1, dtype=np.float32).reshape(S, D)
    c2 = np.asarray(clip2, dtype=np.float32).reshape(S, D)
    sq_a = np.einsum("ij,ij->i", c2, c2, dtype=np.float64)  # rows (x)
    sq_b = np.einsum("ij,ij->i", c1, c1, dtype=np.float64)  # cols (y)
    matrix = -((sq_a[:, None] + sq_b[None, :] - 2.0 * gram) / D) * SCALE
    # diagonal means: row x, col y contributes to diagonal o = y - x
    pdiag = np.zeros(2 * S - 1, dtype=np.float64)
    i = np.arange(S)
    col = (S - 1) - i[:, None] + i[None, :]
    np.add.at(pdiag, col, matrix)
    counts = np.concatenate([np.arange(1, S), np.arange(S, 0, -1)]).astype(
        np.float64
    )
    res = pdiag / counts
    return res[S // 2 - 1 : (S * 3) // 2].astype(np.float32)


def kernel(clip1: np.ndarray, clip2: np.ndarray, **_ignored) -> np.ndarray:
    from concourse.bass_utils import run_bass_kernel_spmd

    in_maps = _shards(clip1, clip2)
    nc = _get_nc()
    res = run_bass_kernel_spmd(nc, in_maps, core_ids=list(range(N_CORES)))
    return _combine(res.results, clip1, clip2)


# revision 19
# speedup vs baseline: 1.0294x; 1.0294x over previous
"""Trainium2 Bass kernel for nn_Classification2 (histogram_binning).

matrix[x, y] = -mean((clip1[y] - clip2[x])**2) * 1e13 over D = 3*224*224
             = -(SCALE/D) * (||a_x||^2 + ||b_y||^2 - 2 a_x.b_y)
output[k]    = mean of matrix over diagonals y - x = k - 64, k in [0, 129)

Strategy: data-parallel over D across 8 NeuronCores. The device computes ONLY
the gram partials a@b^T (the O(S^2 D) part); the O(S D) squared norms come
from the full-precision f32 inputs on the host, and the O(S^2) diagonal
binning also runs on the host.

Inputs stream as fp8 e4m3 (1 B/elem; values are N(0,1), |x| < 6 << 240, no
clipping needed). e4m3 noise averages out over the >=64-element diagonal
means: measured end-to-end rel_l2 ~ 1e-5 (gate 2e-2).

v3 vs the tile-based v1 (28.3us): RAW bass, no TileContext. The walrus
compiler appends a fixed epilogue [CoreBarrier; ~51 semaphore clears per
engine, one EVENT_SEMAPHORE each; CoreBarrier; NOTIFY]. Tile's end-of-kernel
drain + barriers + RANGE_CLEAR (~1.4us) sat between the write-back receipt
and that storm; raw bass drops them, starts the input stream ~0.6us earlier
(no tile-entry barrier), and balances the two HWDGE rings so neither idles
(v1 put 62% of bytes on the sync ring).

Robustness: each engine CLEARS the semaphores it waits on as its first
instructions. Initial sem state is normally all-zero (every walrus NEFF ends
by zeroing all 256), but a hiccuped/aborted prior execution can leave
garbage that would let waits pass early and read stale SBUF (observed as
intermittent NaN/stale grams under NTFF profiling). The clears are safe:
every producer's first increment trails the consumer's clear by multiple us
(first chunk DMA completes ~3us after PE clears its wait sems).

The PE HAM activity monitor clock-gates the array to 1.2 GHz after ~1us of
idleness, which would also halve the PE sequencer's throughput through its
~51-clear share of the walrus storm (115 ns/clear throttled vs ~57 warm).
Dependency-free warm-up matmuls bridge t0 -> first chunk, pace fillers
bridge inter-chunk gaps, and N_TAIL post-stream fillers keep the clock at
2.4 GHz from the last real matmul until the pre-storm CoreBarrier (gated by
the write-back's HBM receipt on Sync).

Engine roles (program order per engine):
  sync   : issue 6 input-chunk DMAs (76 f)
  scalar : clear sem_cp; issue 5 input-chunk DMAs (71 f); wait sem_cp;
           issue the 64 KB gram write-back
  tensor : clear sem_sync/sem_scal; 15 warm matmuls (uninitialized SBUF -
           values are irrelevant, ps_w is never read); per chunk: wait
           chunk sem, matmuls, fillers; N_TAIL tail fillers
  vector : clear sem_pe; wait sem_pe; DVE-copy PSUM->SBUF
  gpsimd : unused (its share of the walrus storm runs during the stream)
"""

import sys
from contextlib import ExitStack

sys.path.insert(0, "/opt/trn_rl_repo")

import numpy as np

S = 128
D = 150528  # 3*224*224
N_CORES = 8
DC = D // N_CORES  # 18816 d-values per core
F = DC // S  # 147 contraction chunks of K=128
SCALE = 1.0e13

# Chunk layout: (ring, nf) in global (f-consecutive) order. Ring 0 = sync,
# ring 1 = scalar. The scalar (qAct) ring runs ~0.5-1us behind at equal
# bytes, so it gets 5 fewer f. Both rings end on a small chunk so the two
# final SBUF-receipt latencies overlap instead of serializing.
CHUNKS = [
    (0, 16), (1, 14),
    (0, 20), (1, 20),
    (0, 20), (1, 20),
    (0, 14), (1, 11),
    (0, 6), (1, 4),
    (0, 2),
]
assert sum(nf for _, nf in CHUNKS) == F
assert sum(nf for r, nf in CHUNKS if r == 0) == 78
# Pace fillers: the PE consumes a chunk faster than DMA delivers the next,
# and idle lumps > ~1us let the HAM activity monitor re-throttle the PE
# clock to 1.2 GHz mid-stream.
FILLER = [0, 0, 2, 2, 2, 2, 2, 2, 1, 1, 0]
N_WARM = 15  # garbage matmuls at t0: bridge PE activity until chunk 0 lands
N_TAIL = 13  # post-stream fillers: keep the PE at 2.4 GHz until it reaches
            # the pre-storm CoreBarrier (~2us after the last real matmul),
            # so its ~51 storm clears run at ~58 ns instead of 116 throttled

_NC_CACHE = {}


def _build():
    import concourse.bacc as bacc
    import concourse.mybir as mybir

    f32 = mybir.dt.float32
    fp8 = mybir.dt.float8e4

    nc = bacc.Bacc(num_devices=N_CORES)

    ba_in = nc.dram_tensor("ba", [S, F * 256], fp8, kind="ExternalInput")
    out_t = nc.dram_tensor("out", [S * S], f32, kind="ExternalOutput")

    with (
        nc.sbuf_tensor([S, F * 256], fp8) as ba_sb,
        nc.sbuf_tensor([S, 256], fp8) as warm_sb,
        nc.sbuf_tensor([S, S], f32) as g_sb,
        nc.psum_tensor([S, 256], f32) as ps_w,
        nc.psum_tensor([S, S], f32) as ps,
        ExitStack() as stack,
    ):
        # One semaphore PER CHUNK, waited at >=16. A single per-ring sem
        # with cumulative >=16k waits is UNSOUND: the +16 per DMA comes
        # from 16 independent SDMA engines, so a fast engine racing ahead
        # through later chunks can push the sem past 16k while a slow
        # engine still owes bytes of chunk k (observed as intermittent
        # stale-partition grams, worse under NTFF profiling's asymmetric
        # engine load). Tile's DMAHW0-7 lane rotation exists for this
        # reason.
        chunk_sems = [
            stack.enter_context(nc.semaphore(name=f"sem_chunk{ci}"))
            for ci in range(len(CHUNKS))
        ]
        sem_pe = stack.enter_context(nc.semaphore(name="sem_pe"))
        sem_cp = stack.enter_context(nc.semaphore(name="sem_cp"))
        sem_out = stack.enter_context(nc.semaphore(name="sem_out"))

        # Self-heal: every engine zeroes the sems it will wait on, before
        # any producer can have incremented them (initial state is all-zero
        # after any clean NEFF, but a hiccuped prior execution can leave
        # garbage; first chunk completion trails these clears by ~3us).
        nc.scalar.sem_clear(sem_cp)
        for s in chunk_sems:
            nc.tensor.sem_clear(s)
        nc.vector.sem_clear(sem_pe)

        # All input chunk DMAs issued up-front; each HWDGE ring drains its
        # FIFO in order while the issuing engine moves on.
        f0 = 0
        for ci, (ring, nf) in enumerate(CHUNKS):
            sl = slice(f0 * 256, (f0 + nf) * 256)
            eng = nc.sync if ring == 0 else nc.scalar
            eng.dma_start(ba_sb[:, sl], ba_in[:, sl]).then_inc(
                chunk_sems[ci], 16
            )
            f0 += nf

        # PE warm-up on whatever bytes sit in warm_sb (NaNs land in the
        # scratch ps_w bank, which is never read).
        for _ in range(N_WARM):
            nc.tensor.matmul(
                ps_w[:, :], warm_sb[:, 0:S], warm_sb[:, :], start=True, stop=True
            )

        f0 = 0
        for ci, (ring, nf) in enumerate(CHUNKS):
            nc.tensor.wait_ge(chunk_sems[ci], 16)
            for j in range(nf):
                f = f0 + j
                base = f * 256
                mm = nc.tensor.matmul(
                    ps[:, :],
                    ba_sb[:, base + S : base + 256],  # lhsT = A_f (clip2)
                    ba_sb[:, base : base + S],  # rhs = B_f (clip1)
                    start=(f == 0),
                    stop=(f == F - 1),
                )
                if f == F - 1:
                    mm.then_inc(sem_pe, 1)
            for _ in range(FILLER[ci]):
                nc.tensor.matmul(
                    ps_w[:, :], warm_sb[:, 0:S], warm_sb[:, :],
                    start=True, stop=True,
                )
            f0 += nf
        for _ in range(N_TAIL):
            nc.tensor.matmul(
                ps_w[:, :], warm_sb[:, 0:S], warm_sb[:, :],
                start=True, stop=True,
            )

        # Evacuate on DVE, write back on the ACT ring. Nobody waits for the
        # write-back's HBM receipt: NRT quiesces the DMA queues at NEFF end
        # (validated empirically - stale reads would return the donated
        # all-zero output buffer), and skipping the wait lets the walrus
        # semaphore-clear storm overlap the ~1.4us receipt instead of
        # serializing after it.
        nc.vector.wait_ge(sem_pe, 1)
        nc.vector.tensor_copy(g_sb[:, :], ps[:, :]).then_inc(sem_cp, 1)
        nc.scalar.wait_ge(sem_cp, 1)
        nc.scalar.dma_start(
            out_t[0 : S * S].rearrange("(p y) -> p y", p=S), g_sb[:, :]
        ).then_inc(sem_out, 16)  # walrus codegen requires a sem update; no waiter

    nc.finalize()
    return nc


def _get_nc():
    if "nc" not in _NC_CACHE:
        _NC_CACHE["nc"] = _build()
    return _NC_CACHE["nc"]


def _shards(clip1: np.ndarray, clip2: np.ndarray):
    """Per-core fp8 [S, F*256] tensors: cols [B_f | A_f] per f, where
    value (p, f, x) = clip[x, d0 + f*128 + p]."""
    import ml_dtypes

    fp8 = ml_dtypes.float8_e4m3
    c1 = np.asarray(clip1, dtype=np.float32).reshape(S, D).astype(fp8)
    c2 = np.asarray(clip2, dtype=np.float32).reshape(S, D).astype(fp8)
    maps = []
    for c in range(N_CORES):
        sl = slice(c * DC, (c + 1) * DC)
        bt = c1[:, sl].reshape(S, F, S).transpose(2, 1, 0)  # [p, f, y] moving
        at = c2[:, sl].reshape(S, F, S).transpose(2, 1, 0)  # [p, f, x] weights
        ba = np.empty((S, F, 256), dtype=fp8)
        ba[:, :, 0:S] = bt
        ba[:, :, S:256] = at
        maps.append({"ba": ba.reshape(S, F * 256)})
    return maps


def _combine(results, clip1: np.ndarray, clip2: np.ndarray) -> np.ndarray:
    gram = np.zeros((S, S), dtype=np.float64)
    for r in results:
        gram += np.asarray(r["out"], dtype=np.float64).reshape(S, S)
    c1 = np.asarray(clip1, dtype=np.float32).reshape(S, D)
    c2 = np.asarray(clip2, dtype=np.float32).reshape(S, D)
    sq_a = np.einsum("ij,ij->i", c2, c2, dtype=np.float64)  # rows (x)
    sq_b = np.einsum("ij,ij->i", c1, c1, dtype=np.float64)  # cols (y)
    matrix = -((sq_a[:, None] + sq_b[None, :] - 2.0 * gram) / D) * SCALE
    # diagonal means: row x, col y contributes to diagonal o = y - x
    pdiag = np.zeros(2 * S - 1, dtype=np.float64)
    i = np.arange(S)
    col = (S - 1) - i[:, None] + i[None, :]
    np.add.at(pdiag, col, matrix)
    counts = np.concatenate([np.arange(1, S), np.arange(S, 0, -1)]).astype(
        np.float64
    )
    res = pdiag / counts
    return res[S // 2 - 1 : (S * 3) // 2].astype(np.float32)


def kernel(clip1: np.ndarray, clip2: np.ndarray, **_ignored) -> np.ndarray:
    from concourse.bass_utils import run_bass_kernel_spmd

    in_maps = _shards(clip1, clip2)
    nc = _get_nc()
    res = run_bass_kernel_spmd(nc, in_maps, core_ids=list(range(N_CORES)))
    return _combine(res.results, clip1, clip2)
